# revision 22
# baseline (speedup 1.0000x reference)
"""Causal local multi-head self-conv-attention Trainium2 kernel.

Problem: nn_CausalLocalMultiHeadSelfConvAttention (B=2, C=512, T=2048,
window W=33, depthwise conv K=3, fc_out over channels).

Sharding: 8 shards = batch(2) x T-chunks(4 x 512). Each core computes its
full output chunk independently: the depthwise conv needs a 34-element left
halo of x and the attention window a 32-element left halo of k/v/mask, both
of which are sliced host-side, so no collectives are needed. fc weights are
replicated.

Per-core math (channels in partitions, 4 tiles of 128):
  q8 = dwconv(x, wq/8)      (scale 1/8 pre-folded host-side; exact in fp32)
  k  = dwconv(x, wk), v = dwconv(x, wv)
  s[c,w,t] = q8[c,t] * k[c,t+w]      for w in 0..32  (t+w indexes the
             halo'd k buffer, i.e. absolute position t0+t-32+w)
  e = exp(s)                (softmax shift dropped: softmax is shift
                             invariant and |s| < ~6 so fp32 exp is safe)
  denom[c,t] = sum_w e * m[t+w]      (m = {0,1} mask, replicated across
  numer[c,t] = sum_w e * (v*m)[t+w]   partitions; masked taps contribute 0
                                      exactly, matching the reference's
                                      exp(-1e9 - max) == 0 in fp32)
  attn = numer / denom
  out  = fc_w @ attn + fc_b          (PE matmul, contraction over channels)
"""

import sys

import numpy as np

for _p in ("/opt/trn_rl_repo",):
    if _p not in sys.path:
        sys.path.insert(0, _p)

import concourse.bass as bass
import concourse.tile as tile
from concourse import bacc
from concourse import mybir
from concourse.ap import AP

P = 128          # partitions per channel tile
NCT = 4          # channel tiles (C = 512)
TC = 512         # per-core time chunk
W = 33           # attention window
WC = 8           # w-chunk size for the main loop (4 chunks of 8 + tail w=32)
XH = TC + 34     # x halo: conv(2) + window(32)
KH = TC + 32     # k/v/mask halo
F32 = mybir.dt.float32
MULT = mybir.AluOpType.mult
ADD = mybir.AluOpType.add


def _sliding(t, w0, wc, d2=None):
    """View of a [P, F] tile as [P, wc, TC] (or [P, d2, wc, TC]) where
    element (p, w, i) = t[p, w0 + w + i] (and the optional leading free dim
    steps by `d2` columns: element (p, d, w, i) = t[p, d*d2 + w0 + w + i])."""
    base = t[:, w0:w0 + TC]
    pstep, pcount = base.ap[0]
    free = [[1, wc], [1, TC]]
    if d2 is not None:
        free = [[d2, 2]] + free
    return AP(base.tensor, base.offset, [[pstep, pcount]] + free)


def build_program():
    nc = bacc.Bacc("TRN2")

    # x and the 9 conv-tap weights ride one DRAM tensor so each channel
    # tile is loaded by a single DMA (one semaphore): TRN2 compute
    # instructions accept only one sync wait.
    xw_d = nc.dram_tensor("xw", [P * NCT, XH + 9], F32, kind="ExternalInput")
    m_d = nc.dram_tensor("m", [1, KH], F32, kind="ExternalInput")
    fwt_d = nc.dram_tensor("fwt", [P * NCT, P * NCT], F32, kind="ExternalInput")
    fb_d = nc.dram_tensor("fb", [P * NCT, 1], F32, kind="ExternalInput")
    out_d = nc.dram_tensor("out", [P * NCT, TC], F32, kind="ExternalOutput")

    with tile.TileContext(nc) as tc:
        with (
            tc.tile_pool(name="inp", bufs=2) as inp,
            tc.tile_pool(name="conv", bufs=2) as convp,
            tc.tile_pool(name="shared", bufs=1) as shared,
            tc.tile_pool(name="work", bufs=2) as work,
            tc.tile_pool(name="work1", bufs=1) as work1,
            tc.tile_pool(name="tree", bufs=2) as tree,
            tc.tile_pool(name="accp", bufs=2) as accp,
            tc.tile_pool(name="fc", bufs=1) as fcp,
            tc.tile_pool(name="outp", bufs=2) as outp,
            tc.tile_pool(name="psum", bufs=1, space="PSUM") as psum,
        ):
            # mask broadcast to all partitions; VM2 = [m | v*m] per ctile
            m128 = shared.tile([P, KH], F32)
            nc.sync.dma_start(m128[:], m_d[0].partition_broadcast(P))

            fwt_sb = fcp.tile([P, NCT, P * NCT], F32)
            nc.sync.dma_start(
                fwt_sb[:], fwt_d[:].rearrange("(n p) o -> p n o", p=P))
            fb_raw = fcp.tile([P, NCT], F32)
            nc.sync.dma_start(fb_raw[:], fb_d[:, 0].rearrange("(n p) -> p n", p=P))
            fb_sb = fcp.tile([P, NCT], F32)
            nc.scalar.copy(fb_sb[:], fb_raw[:])

            attn_all = fcp.tile([P, NCT, TC], F32)

            q8s, k_sbs, vm2s = [], [], []
            for ci in range(NCT):
                xw_sb = inp.tile([P, XH + 9], F32)
                nc.sync.dma_start(xw_sb[:], xw_d[ci * P:(ci + 1) * P, :])
                x_sb = xw_sb
                wq_sb = xw_sb[:, XH:XH + 3]
                wk_sb = xw_sb[:, XH + 3:XH + 6]
                wv_sb = xw_sb[:, XH + 6:XH + 9]

                # --- depthwise causal conv (3 taps) ---
                # Tap products on ACT (scale is a per-partition AP);
                # accumulation on DVE. ACT has slack and this keeps
                # multi-sync-wait structs off the DVE stream (TRN2 compute
                # ops accept one sync wait).
                tap = convp.tile([P, 3, KH], F32, tag="tap")
                for j in range(3):
                    nc.scalar.mul(tap[:, j, :], x_sb[:, j:j + KH],
                                  wq_sb[:, j:j + 1])
                q8 = shared.tile([P, TC], F32, tag=f"q8_{ci}")
                nc.vector.tensor_tensor(q8[:], tap[:, 0, 32:32 + TC],
                                        tap[:, 1, 32:32 + TC], ADD)
                nc.vector.tensor_tensor(q8[:], q8[:],
                                        tap[:, 2, 32:32 + TC], ADD)

                for j in range(3):
                    nc.scalar.mul(tap[:, j, :], x_sb[:, j:j + KH],
                                  wk_sb[:, j:j + 1])
                k_sb = shared.tile([P, KH], F32, tag=f"k_{ci}")
                nc.vector.tensor_tensor(k_sb[:], tap[:, 0, :], tap[:, 1, :], ADD)
                nc.vector.tensor_tensor(k_sb[:], k_sb[:], tap[:, 2, :], ADD)

                # VM2 = [m128 | v*m128] (two KH-wide halves of one tile)
                vm2 = shared.tile([P, 2 * KH], F32, tag=f"vm2_{ci}")
                nc.vector.tensor_copy(vm2[:, 0:KH], m128[:])
                v_half = vm2[:, KH:2 * KH]
                for j in range(3):
                    nc.scalar.mul(tap[:, j, :], x_sb[:, j:j + KH],
                                  wv_sb[:, j:j + 1])
                nc.vector.tensor_tensor(v_half, tap[:, 0, :], tap[:, 1, :], ADD)
                nc.vector.tensor_tensor(v_half, v_half, tap[:, 2, :], ADD)
                nc.vector.tensor_tensor(v_half, v_half, m128[:], MULT)
                q8s.append(q8)
                k_sbs.append(k_sb)
                vm2s.append(vm2)

            psums = [psum.tile([P, TC], F32, tag=f"ps{oi}", name=f"ps{oi}")
                     for oi in range(NCT)]
            for ci in range(NCT):
                q8, k_sb, vm2 = q8s[ci], k_sbs[ci], vm2s[ci]

                # --- windowed softmax accumulation ---
                # acc2[:, 0, :] = denom, acc2[:, 1, :] = numer
                acc2 = accp.tile([P, 2, TC], F32)
                for wi, w0 in enumerate(range(0, W - 1, WC)):
                    s = work.tile([P, WC, TC], F32, tag="s")
                    nc.gpsimd.tensor_tensor(
                        s[:], q8[:].unsqueeze(1).broadcast_to([P, WC, TC]),
                        _sliding(k_sb, w0, WC), MULT)
                    nc.scalar.activation(s[:], s[:],
                                         mybir.ActivationFunctionType.Exp)
                    uy = work1.tile([P, 2, WC, TC], F32, tag="uy")
                    nc.vector.tensor_tensor(
                        uy[:], s[:].unsqueeze(1).broadcast_to([P, 2, WC, TC]),
                        _sliding(vm2, w0, WC, d2=KH), MULT)
                    t1 = tree.tile([P, 2, WC // 2, TC], F32, tag="t1")
                    nc.vector.tensor_tensor(t1[:], uy[:, :, 0:WC:2, :],
                                            uy[:, :, 1:WC:2, :], ADD)
                    t2 = work1.tile([P, 2, WC // 4, TC], F32, tag="t2")
                    nc.gpsimd.tensor_tensor(t2[:], t1[:, :, 0:WC // 2:2, :],
                                            t1[:, :, 1:WC // 2:2, :], ADD)
                    if wi == 0:
                        nc.gpsimd.tensor_tensor(acc2[:], t2[:, :, 0, :],
                                                t2[:, :, 1, :], ADD)
                    else:
                        t3 = tree.tile([P, 2, TC], F32, tag="t3")
                        nc.gpsimd.tensor_tensor(t3[:], t2[:, :, 0, :],
                                                t2[:, :, 1, :], ADD)
                        nc.vector.tensor_tensor(acc2[:], acc2[:], t3[:], ADD)

                # tail tap w = 32
                s32 = work1.tile([P, TC], F32, tag="s32")
                nc.gpsimd.tensor_tensor(s32[:], q8[:], k_sb[:, 32:32 + TC], MULT)
                nc.scalar.activation(s32[:], s32[:],
                                     mybir.ActivationFunctionType.Exp)
                uy32 = work1.tile([P, 2, TC], F32, tag="uy32")
                nc.vector.tensor_tensor(
                    uy32[:], s32[:].unsqueeze(1).broadcast_to([P, 2, TC]),
                    _sliding(vm2, 32, 1, d2=KH).squeeze(2), MULT)
                # +1e-30 keeps fully-masked windows at 0/eps = 0 instead of
                # 0/0 = NaN (those columns are host-fixed afterwards).
                nc.vector.scalar_tensor_tensor(acc2[:], uy32[:], 1e-30,
                                               acc2[:], ADD, ADD)

                rec = work1.tile([P, TC], F32, tag="rec")
                nc.vector.reciprocal_approx_fast(rec[:], acc2[:, 0, :])
                nc.vector.tensor_tensor(attn_all[:, ci, :], acc2[:, 1, :],
                                        rec[:], MULT)

                # fc contribution of this ctile (PE is otherwise idle; doing
                # it here hides all but the last ctile's matmuls)
                for oi in range(NCT):
                    nc.tensor.matmul(
                        psums[oi][:], fwt_sb[:, ci, oi * P:(oi + 1) * P],
                        attn_all[:, ci, :],
                        start=(ci == 0), stop=(ci == NCT - 1))

            for oi in range(NCT):
                o_sb = outp.tile([P, TC], F32)
                nc.scalar.activation(o_sb[:], psums[oi][:],
                                     mybir.ActivationFunctionType.Identity,
                                     bias=fb_sb[:, oi:oi + 1])
                nc.sync.dma_start(out_d[oi * P:(oi + 1) * P, :], o_sb[:])

    nc.finalize()
    return nc


_NC_CACHE = []


def _get_nc():
    if not _NC_CACHE:
        _NC_CACHE.append(build_program())
    return _NC_CACHE[0]


def make_in_maps(x, mask, wq, wk, wv, fc_w, fc_b):
    x = np.asarray(x, dtype=np.float32)          # (2, 512, 2048)
    mask_np = np.asarray(mask)                   # (2, 1, 2048) int32
    wq = np.asarray(wq, dtype=np.float32)        # (512, 1, 3)
    wk = np.asarray(wk, dtype=np.float32)
    wv = np.asarray(wv, dtype=np.float32)
    fc_w = np.asarray(fc_w, dtype=np.float32)    # (512, 512)
    fc_b = np.asarray(fc_b, dtype=np.float32)    # (512,)

    B, C, T = x.shape
    n_tc = T // TC
    wq8 = np.ascontiguousarray(wq[:, 0, :] / 8.0)   # fold 1/sqrt(D): exact
    wk2 = np.ascontiguousarray(wk[:, 0, :])
    wv2 = np.ascontiguousarray(wv[:, 0, :])
    fwt = np.ascontiguousarray(fc_w.T)
    fb = np.ascontiguousarray(fc_b.reshape(C, 1))
    mask_f = mask_np.astype(np.float32)

    in_maps = []
    for core in range(B * n_tc):
        b, tci = divmod(core, n_tc)
        t0 = tci * TC
        xw = np.zeros((C, XH + 9), np.float32)
        lo = t0 - 34
        pad = max(0, -lo)
        xw[:, pad:XH] = x[b, :, lo + pad:t0 + TC]
        xw[:, XH:XH + 3] = wq8
        xw[:, XH + 3:XH + 6] = wk2
        xw[:, XH + 6:XH + 9] = wv2
        ms = np.zeros((1, KH), np.float32)
        lo = t0 - 32
        pad = max(0, -lo)
        ms[:, pad:] = mask_f[b, 0, lo + pad:t0 + TC]
        in_maps.append({"xw": xw, "m": ms, "fwt": fwt, "fb": fb})
    return in_maps


def kernel(x, mask, wq, wk, wv, fc_w, fc_b):
    from concourse import bass_utils

    mask_np = np.asarray(mask)
    in_maps = make_in_maps(x, mask, wq, wk, wv, fc_w, fc_b)
    B, C, T = np.asarray(x).shape
    n_tc = T // TC
    res = bass_utils.run_bass_kernel_spmd(_get_nc(), in_maps, list(range(8)))
    out = np.empty((B, C, T), np.float32)
    for core in range(8):
        b, tci = divmod(core, n_tc)
        out[b, :, tci * TC:(tci + 1) * TC] = res.results[core]["out"]
    _fix_dead_windows(out, x, mask_np, wq, wk, wv, fc_w, fc_b)
    return out, mask_np


def _fix_dead_windows(out, x, mask, wq, wk, wv, fc_w, fc_b):
    """Columns whose whole 33-tap window is masked: the reference's softmax
    over all -1e9 scores is uniform (1/33 each), so attn = mean of the
    zero-padded v window. The device path yields 0/0 there; recompute those
    few columns exactly on the host (the window-33 left pad means this
    happens when the first mask entries are 0, ~1.5 columns per sequence)."""
    x = np.asarray(x, np.float32)
    mask = np.asarray(mask)
    B, C, T = x.shape
    mp = np.pad(mask[:, 0, :], ((0, 0), (32, 0)))
    wins = np.lib.stride_tricks.sliding_window_view(mp, 33, axis=1).sum(-1)
    dead = np.argwhere(wins == 0)
    if not len(dead):
        return
    wv2 = np.asarray(wv, np.float32)[:, 0, :]
    fc_w = np.asarray(fc_w, np.float32)
    fc_b = np.asarray(fc_b, np.float32)
    xp = np.pad(x, ((0, 0), (0, 0), (2, 0)))
    for b, t in dead:
        js = np.arange(max(0, t - 32), t + 1)
        # v[c, j] = sum_i xp[b, c, j + i] * wv[c, i]
        vseg = np.einsum("cji,ci->cj", np.stack(
            [xp[b, :, j:j + 3] for j in js], axis=1), wv2)
        attn = vseg.sum(axis=1) / 33.0
        out[b, :, t] = fc_w @ attn + fc_b

